# revision 1
# baseline (speedup 1.0000x reference)
"""GAT (graph attention) forward on 8 TRN2 NeuronCores, Bass/Tile.

Sharding: target nodes (rows of the output) split into 8 blocks of 512.
Each core redundantly computes the projected features h for ALL nodes
(cheap: one K=128 matmul chain) and then its own 512-row slice of the
attention + aggregation + skip + ELU.  No collectives.

Score layout trick: scores are built in [m, n] layout (source node m on
partitions, local target n on the free dim) so the unnormalized attention
tile is already transposed for the PE aggregation matmul
    out_ext^T[f, n] = sum_m h_ext[m, f] * exp_scores[m, n]
where h_ext has a ones-column appended (f = 64) so the softmax
denominator falls out of the same matmul.  Softmax skips the max-shift:
logits are O(10), masked entries hold -1e9 and underflow exp -> exactly 0.

The mask is transposed on the host per core so the device only ever does
contiguous row DMA.

All DMA goes through SWDGE (gpsimd.dma_start) and PE-read constants are
packed into one DRAM parameter: the S3_LW (weight-load) instruction can
carry only one semaphore wait, so every matmul must depend on at most
one producer semaphore.  Cheap PE warmup ops absorb first-touch waits.
"""

import numpy as np
from contextlib import ExitStack

import concourse.bass as bass
import concourse.mybir as mybir
from concourse.tile import TileContext
from concourse.masks import make_identity
from concourse.bass_utils import run_bass_kernel_spmd

F32 = mybir.dt.float32
AF = mybir.ActivationFunctionType
OP = mybir.AluOpType

N, FIN, H, FOUT = 4096, 128, 4, 64
G = H * FOUT
NCORES = 8
NLOC = N // NCORES          # local target rows per core
NCH = N // 128              # source (m) chunks
LCH = NLOC // 128           # local output row chunks


def build_program(n=N, h_heads=H, fout=FOUT, nloc=NLOC):
    g = h_heads * fout
    nch = n // 128
    lch = nloc // 128
    he = fout + 1            # h_ext columns (ones col at index fout)
    # cpack: xT | wproj | wsc | wskip | xTloc | biasb  (one DMA, one sem)
    cw = n + g + 2 * h_heads + g + nloc + g

    nc = bass.Bass()
    d_cpack = nc.declare_dram_parameter("cpack", [128, cw], F32, isOutput=False)
    d_maskT = nc.declare_dram_parameter("maskT", [n, nloc], F32, isOutput=False)
    d_out = nc.declare_dram_parameter("out", [nloc, g], F32, isOutput=True)

    with TileContext(nc) as tc, ExitStack() as ctx:
        cp = ctx.enter_context(tc.tile_pool(name="const", bufs=1))
        sb_cpack = cp.tile([128, cw], F32, tag="cpack")
        o = 0
        sb_xT = sb_cpack[:, o:o + n]; o += n
        sb_wproj = sb_cpack[:, o:o + g]; o += g
        sb_wsc = sb_cpack[:, o:o + 2 * h_heads]; o += 2 * h_heads
        sb_wskip = sb_cpack[:, o:o + g]; o += g
        sb_xTloc = sb_cpack[:, o:o + nloc]; o += nloc
        sb_biasb = sb_cpack[:, o:o + g]; o += g
        sb_ones = cp.tile([128, 128], F32, tag="ones")
        sb_id = cp.tile([128, 128], F32, tag="ident")
        sb_mask = cp.tile([128, nch * nloc], F32, tag="mask")
        sb_h = cp.tile([128, nch * h_heads * he], F32, tag="hext")
        sb_stgt = cp.tile([128, nch * h_heads], F32, tag="stgt")
        sb_ssrc = cp.tile([128, h_heads * nloc], F32, tag="ssrc")

        nc.gpsimd.dma_start(out=sb_cpack[:], in_=d_cpack[:])
        nc.vector.memset(sb_ones[:], 1.0)
        make_identity(nc, sb_id[:])
        for j in range(nch):
            nc.gpsimd.dma_start(out=sb_mask[:, j * nloc:(j + 1) * nloc],
                                in_=d_maskT[j * 128:(j + 1) * 128, :])

        # h_ext view: [128, nch*h, he]; chunk (j, head) at index j*h + head
        hv = sb_h[:].rearrange("p (c w) -> p c w", w=he)
        nc.vector.memset(hv[:, :, fout:fout + 1], 1.0)

        # ---- phase 1: h = x @ proj (all heads at once) + s_tgt -------------
        with tc.tile_pool(name="ps1", bufs=2, space="PSUM") as ps1:
            for j in range(nch):
                ph = ps1.tile([128, g + h_heads], F32, tag="ph")
                lhs = sb_xT[:, j * 128:(j + 1) * 128]
                nc.tensor.matmul(ph[:, 0:g], lhs, sb_wproj, start=True, stop=True)
                nc.tensor.matmul(ph[:, g:g + h_heads], lhs,
                                 sb_wsc[:, h_heads:2 * h_heads], start=True, stop=True)
                src_h = ph[:, 0:g].rearrange("p (hh f) -> p hh f", f=fout)
                nc.vector.tensor_copy(hv[:, j * h_heads:(j + 1) * h_heads, 0:fout], src_h)
                nc.vector.tensor_copy(sb_stgt[:, j * h_heads:(j + 1) * h_heads],
                                      ph[:, g:g + h_heads])

            # PE warmups: absorb first-touch semaphore waits so that the
            # hot-loop matmuls each carry a single wait (S3_LW limit).
            pscr = ps1.tile([128, 32], F32, tag="pscr")
            nc.tensor.matmul(pscr[0:he, 0:1], hv[:, 0, :], sb_ones[:, 0:1],
                             start=True, stop=True)
            pscr2 = ps1.tile([128, 32], F32, tag="pscr2")
            nc.tensor.transpose(pscr2[0:32, 0:32], sb_id[0:32, 0:32],
                                sb_id[0:32, 0:32])

            # ---- phase 2: s_src broadcast tiles [128, nloc] per head -------
            for hh in range(h_heads):
                tmp = cp.tile([128, nloc], F32, tag="bctmp")
                nc.vector.tensor_scalar(tmp[:], sb_xTloc[:], sb_wsc[:, hh:hh + 1],
                                        None, OP.mult)
                pb = ps1.tile([128, nloc], F32, tag="pb")
                nc.tensor.matmul(pb[:], sb_ones[:], tmp[:], start=True, stop=True)
                nc.scalar.copy(sb_ssrc[:, hh * nloc:(hh + 1) * nloc], pb[:])

        # ---- phase 3: attention main loop ---------------------------------
        po = []
        pso = ctx.enter_context(tc.tile_pool(name="pso", bufs=1, space="PSUM"))
        for hh in range(h_heads):
            po.append(pso.tile([128, nloc], F32, tag=f"po{hh}", name=f"po{hh}"))

        with tc.tile_pool(name="work", bufs=4) as wp:
            for hh in range(h_heads):
                ssrc = sb_ssrc[:, hh * nloc:(hh + 1) * nloc]
                for j in range(nch):
                    zt = wp.tile([128, nloc], F32, tag="zt")
                    # z = s_src[n] + s_tgt[m]  (ACT, bias = per-partition scalar)
                    nc.scalar.activation(zt[:], ssrc, AF.Identity,
                                         bias=sb_stgt[:, j * h_heads + hh:
                                                      j * h_heads + hh + 1])
                    # leaky_relu(z, 0.2) = max(z, 0.2 z)
                    lt = wp.tile([128, nloc], F32, tag="lt")
                    nc.vector.tensor_scalar(lt[:], zt[:], 0.2, None, OP.mult)
                    nc.vector.tensor_tensor(zt[:], zt[:], lt[:], OP.max)
                    # + mask (0 / -1e9), then exp
                    nc.vector.tensor_tensor(zt[:], zt[:],
                                            sb_mask[:, j * nloc:(j + 1) * nloc], OP.add)
                    et = wp.tile([128, nloc], F32, tag="et")
                    nc.scalar.activation(et[:], zt[:], AF.Exp)
                    nc.tensor.matmul(po[hh][0:he, :],
                                     hv[:, j * h_heads + hh, :], et[:],
                                     start=(j == 0), stop=(j == nch - 1))

            # ---- phase 4/5: normalize, transpose, skip, bias, ELU ---------
            pon = []
            for hh in range(h_heads):
                pos = cp.tile([128, nloc], F32, tag=f"pos{hh}", name=f"pos{hh}")
                nc.scalar.copy(pos[0:he, :], po[hh][0:he, :])
                pon.append(pos)

        with tc.tile_pool(name="fin", bufs=2) as fp, \
             tc.tile_pool(name="psf", bufs=2, space="PSUM") as psf:
            for li in range(lch):
                af = fp.tile([128, g], F32, tag="af")
                for hh in range(h_heads):
                    pt = psf.tile([128, he], F32, tag="pt")
                    nc.tensor.transpose(pt[0:128, 0:he],
                                        pon[hh][0:he, li * 128:(li + 1) * 128],
                                        sb_id[0:he, 0:he])
                    rcp = fp.tile([128, 1], F32, tag="rcp")
                    nc.vector.reciprocal(rcp[:], pt[:, fout:fout + 1])
                    nc.vector.tensor_scalar(af[:, hh * fout:(hh + 1) * fout],
                                            pt[:, 0:fout], rcp[:], None, OP.mult)
                # skip connection: x_loc_chunk @ skip_w.T  (+ bias)
                pskip = psf.tile([128, g], F32, tag="pskip")
                nc.tensor.matmul(pskip[:], sb_xTloc[:, li * 128:(li + 1) * 128],
                                 sb_wskip, start=True, stop=True)
                nc.vector.tensor_tensor(af[:], af[:], pskip[:], OP.add)
                nc.vector.tensor_tensor(af[:], af[:], sb_biasb[:], OP.add)
                # ELU(z) = max(z,0) + exp(min(z,0)) - 1
                mn = fp.tile([128, g], F32, tag="mn")
                nc.vector.tensor_scalar(mn[:], af[:], 0.0, None, OP.min)
                ex = fp.tile([128, g], F32, tag="ex")
                nc.scalar.activation(ex[:], mn[:], AF.Exp)
                nc.vector.tensor_scalar(af[:], af[:], 0.0, None, OP.max)
                nc.vector.tensor_tensor(af[:], af[:], ex[:], OP.add)
                nc.vector.tensor_scalar(af[:], af[:], -1.0, None, OP.add)
                nc.gpsimd.dma_start(out=d_out[li * 128:(li + 1) * 128, :], in_=af[:])

    _split_multi_waits(nc)
    return nc


def _split_multi_waits(nc):
    """walrus on this toolchain allows only one semaphore-wait command on
    most compute-engine instructions (S3_LW / S3D3_* structs).  Tile's
    scheduler freely emits 2+.  Move all but one wait onto an injected
    same-engine NoOp right before the offending instruction."""
    skip = (mybir.InstEventSemaphore,)
    k = 0
    for f in nc.m.functions:
        for blk in f.blocks:
            new = []
            for ins in blk.instructions:
                si = getattr(ins, "sync_info", None)
                w = list(si.on_wait) if si is not None and si.on_wait else []
                if len(w) > 1 and not isinstance(ins, skip):
                    for wx in w[:-1]:
                        nop = mybir.InstNoOp(name=f"waitsplit-{k}", ins=[], outs=[])
                        nop.engine = ins.engine
                        nop.sync_info = mybir.SyncInfo(on_wait=[wx], on_update=[])
                        new.append(nop)
                        k += 1
                    ins.sync_info = mybir.SyncInfo(on_wait=w[-1:],
                                                   on_update=list(si.on_update))
                new.append(ins)
            blk.instructions[:] = new


_PROG = None


def _get_prog():
    global _PROG
    if _PROG is None:
        _PROG = build_program()
    return _PROG


def make_in_maps(x, mask, proj_param, score_src, score_tgt, skip_w, bias):
    x = np.asarray(x, np.float32)
    mask = np.asarray(mask, np.float32)
    proj = np.asarray(proj_param, np.float32)
    a_src = np.asarray(score_src, np.float32)[:, :, 0]       # [H, FOUT]
    a_tgt = np.asarray(score_tgt, np.float32)[:, :, 0]
    skip = np.asarray(skip_w, np.float32)
    b = np.asarray(bias, np.float32)

    xT = np.ascontiguousarray(x.T)                           # [128, N]
    wproj = np.ascontiguousarray(proj.transpose(1, 0, 2).reshape(FIN, G))
    w_src = np.einsum('hif,hf->ih', proj, a_src)             # [FIN, H]
    w_tgt = np.einsum('hif,hf->ih', proj, a_tgt)
    wsc = np.concatenate([w_src, w_tgt], 1).astype(np.float32)
    wskip = np.ascontiguousarray(skip.T)                     # [128, G]
    biasb = np.broadcast_to(b[None, :], (128, G)).astype(np.float32)

    in_maps = []
    for c in range(NCORES):
        r0 = c * NLOC
        cpack = np.ascontiguousarray(np.concatenate(
            [xT, wproj, wsc, wskip, xT[:, r0:r0 + NLOC], biasb], axis=1),
            np.float32)
        in_maps.append({
            "cpack": cpack,
            "maskT": np.ascontiguousarray(mask[r0:r0 + NLOC, :].T),
        })
    return in_maps


def run(in_maps, trace=False, **kw):
    res = run_bass_kernel_spmd(_get_prog(), in_maps, list(range(NCORES)),
                               trace=trace, **kw)
    out = np.concatenate([res.results[c]["out"] for c in range(NCORES)], axis=0)
    return out, res


def kernel(x, mask, proj_param, score_src, score_tgt, skip_w, bias):
    in_maps = make_in_maps(x, mask, proj_param, score_src, score_tgt, skip_w, bias)
    out, _ = run(in_maps)
    return out.astype(np.float32)



# revision 13
# speedup vs baseline: 1.7606x; 1.7606x over previous
"""GAT (graph attention) forward on 8 TRN2 NeuronCores, Bass/Tile.

Sharding: target nodes (rows of the output) split into 8 blocks of 512.
Each core redundantly computes the projected features h for ALL nodes
(cheap: one K=128 matmul chain) and then its own 512-row slice of the
attention + aggregation + skip + ELU.  No collectives.

Score factorization trick: with z[m,n] = s_tgt[m] + s_src[n] and
leaky(z) = max(z, 0.2 z),

    exp(leaky(z)) = max(e^z, e^{0.2 z})
                  = e^{0.2 s_src[n]} * max(u_m * w_n, p_m)

where u = e^{s_tgt}, p = e^{0.2 s_tgt}, w = e^{0.8 s_src}.  The leading
per-target factor cancels in the softmax normalization, so the kernel
computes only  et[m,n] = M01[m,n] * max(u_m * w_n, p_m)  per tile:
one DVE tensor_scalar (two per-partition scalars, bf16 -> 4x mode) and
one DVE tensor_tensor multiply with the 0/1 bf16 mask (2x mode).  The
aggregation matmul runs in bf16 (1 col/cycle) with a ones-column
appended to h so the softmax denominator falls out of the same matmul.

Projection/skip/score matmuls run as float32r (full-rate PE, ~tf32).
The mask is transposed and converted to multiplicative 0/1 bf16 on the
host so the device only ever does contiguous row DMA.

All DMA goes through SWDGE (gpsimd.dma_start); the S3_LW (weight-load)
instruction can carry only one semaphore wait, so _split_multi_waits
rewrites any instruction Tile scheduled with 2+ waits.
"""

import numpy as np
from contextlib import ExitStack

import concourse.bass as bass
import concourse.mybir as mybir
from concourse.tile import TileContext
from concourse.masks import make_identity
from concourse.bass_utils import run_bass_kernel_spmd

F32 = mybir.dt.float32
F32R = mybir.dt.float32r
BF16 = mybir.dt.bfloat16
AF = mybir.ActivationFunctionType
OP = mybir.AluOpType

N, FIN, H, FOUT = 4096, 128, 4, 64
G = H * FOUT
NCORES = 8
NLOC = N // NCORES          # local target rows per core
NCH = N // 128              # source (m) chunks
LCH = NLOC // 128           # local output row chunks
HE = FOUT + 1               # h_ext columns (ones col at index FOUT)
XPC = 8                     # phase-1 chunks per xT DMA piece
NXP = NCH // XPC            # xT DMA pieces


def build_program():
    # cpack: xT | wproj | wsc | wskip | xTloc | biasb  (fp32)
    cw = N + G + 2 * H + G + NLOC + G

    nc = bass.Bass()
    d_cpack = nc.declare_dram_parameter("cpack", [128, cw], F32R, isOutput=False)
    d_mask = nc.declare_dram_parameter("mask01", [N, NLOC], BF16, isOutput=False)
    d_sel = nc.declare_dram_parameter("selc", [4, 4 * 128], BF16, isOutput=False)
    d_out = nc.declare_dram_parameter("out", [NLOC, G], F32, isOutput=True)

    with TileContext(nc) as tc, ExitStack() as ctx:
        cp = ctx.enter_context(tc.tile_pool(name="const", bufs=1))
        sb_cpack = cp.tile([128, cw], F32R, tag="cpack")
        o = 0
        xTr = sb_cpack[:, o:o + N]; o += N
        wprojr = sb_cpack[:, o:o + G]; o += G
        wscr = sb_cpack[:, o:o + 2 * H]; o += 2 * H
        wskipr = sb_cpack[:, o:o + G]; o += G
        xTlocr = sb_cpack[:, o:o + NLOC]; o += NLOC
        sb_biasb = sb_cpack[:, o:o + G].bitcast(F32); o += G

        sb_sel = cp.tile([4, 4 * 128], BF16, tag="sel")   # one-hot head rows
        sb_id = cp.tile([128, 128], F32, tag="ident")
        sb_h = cp.tile([128, NCH * H * HE], BF16, tag="hext")
        sb_w = cp.tile([128, H * NLOC], BF16, tag="wbc")      # e^{0.8 s_src}
        sb_wrow = cp.tile([4, NLOC], BF16, tag="wrow")
        sb_u = cp.tile([128, NCH * H], F32, tag="uexp")       # e^{s_tgt}
        sb_p = cp.tile([128, NCH * H], F32, tag="pexp")       # e^{0.2 s_tgt}
        sb_m = [cp.tile([128, NLOC], BF16, tag=f"m{j}", name=f"m{j}")
                for j in range(NCH)]

        # ---- DMA: weights first, then xT pieces, then mask chunks ---------
        nc.gpsimd.dma_start(out=sb_sel[:], in_=d_sel[:])
        nc.gpsimd.dma_start(out=sb_cpack[:, N:cw], in_=d_cpack[:, N:cw])
        for p in range(NXP):
            w0 = p * XPC * 128
            nc.gpsimd.dma_start(out=sb_cpack[:, w0:w0 + XPC * 128],
                                in_=d_cpack[:, w0:w0 + XPC * 128])
        for j in range(NCH):
            nc.gpsimd.dma_start(out=sb_m[j][:],
                                in_=d_mask[j * 128:(j + 1) * 128, :])

        make_identity(nc, sb_id[:])

        # h_ext view: [128, c, HE]; chunk (j, head) at index j*H + head
        hv = sb_h[:].rearrange("p (c w) -> p c w", w=HE)
        nc.vector.memset(hv[:, :, FOUT:FOUT + 1], 1.0)

        # ---- phase 0: b = s_src(local), w = e^{0.8 b} broadcast -----------
        with tc.tile_pool(name="ps0", bufs=1, space="PSUM") as ps0:
            pb = ps0.tile([4, NLOC], F32, tag="pb")
            nc.tensor.matmul(pb[:], wscr[:, 0:H], xTlocr, start=True, stop=True)
            nc.scalar.activation(sb_wrow[:], pb[:], AF.Exp, scale=0.8)
            pwb = ps0.tile([128, NLOC], F32, tag="pwb")
            for hh in range(H):
                nc.tensor.matmul(pwb[:], sb_sel[0:4, hh * 128:(hh + 1) * 128],
                                 sb_wrow[0:4, :], start=True, stop=True)
                nc.scalar.copy(sb_w[:, hh * NLOC:(hh + 1) * NLOC], pwb[:])

        # ---- phase 1: h = x @ proj (all heads), u/p = exp(s_tgt) ----------
        with tc.tile_pool(name="ps1", bufs=3, space="PSUM") as ps1:
            for j in range(NCH):
                ph = ps1.tile([128, G + H], F32, tag="ph")
                lhs = xTr[:, j * 128:(j + 1) * 128]
                nc.tensor.matmul(ph[:, 0:G], lhs, wprojr, start=True, stop=True)
                nc.tensor.matmul(ph[:, G:G + H], lhs, wscr[:, H:2 * H],
                                 start=True, stop=True)
                src_h = ph[:, 0:G].rearrange("p (hh f) -> p hh f", f=FOUT)
                nc.scalar.copy(hv[:, j * H:(j + 1) * H, 0:FOUT], src_h)
                nc.scalar.activation(sb_u[:, j * H:(j + 1) * H],
                                     ph[:, G:G + H], AF.Exp)
                nc.scalar.activation(sb_p[:, j * H:(j + 1) * H],
                                     ph[:, G:G + H], AF.Exp, scale=0.2)

        # ---- phase 3: attention main loop ---------------------------------
        po = []
        pso = ctx.enter_context(tc.tile_pool(name="pso", bufs=1, space="PSUM"))
        for hh in range(H):
            po.append(pso.tile([HE, NLOC], F32, tag=f"po{hh}", name=f"po{hh}"))

        with tc.tile_pool(name="work", bufs=6) as wp:
            for hh in range(H):
                wb = sb_w[:, hh * NLOC:(hh + 1) * NLOC]
                for j in range(NCH):
                    c = j * H + hh
                    t1 = wp.tile([128, NLOC], BF16, tag="t1")
                    nc.vector.tensor_scalar(t1[:], wb, sb_u[:, c:c + 1],
                                            sb_p[:, c:c + 1], OP.mult, OP.max)
                    et = wp.tile([128, NLOC], BF16, tag="et")
                    nc.vector.tensor_tensor(et[:], t1[:], sb_m[j][:], OP.mult)
                    nc.tensor.matmul(po[hh][:], hv[:, c, :], et[:],
                                     start=(j == 0), stop=(j == NCH - 1))

            # copy accumulators out of PSUM so PE can transpose from SBUF
            pon = []
            for hh in range(H):
                pos = cp.tile([HE, NLOC], F32, tag=f"pos{hh}", name=f"pos{hh}")
                nc.scalar.copy(pos[:], po[hh][:])
                pon.append(pos)

        # ---- phase 4: normalize, transpose, skip, bias, ELU ---------------
        with tc.tile_pool(name="fin", bufs=2) as fp, \
             tc.tile_pool(name="psf", bufs=2, space="PSUM") as psf:
            for li in range(LCH):
                af = fp.tile([128, G], F32, tag="af")
                for hh in range(H):
                    pt = psf.tile([128, HE], F32, tag="pt")
                    nc.tensor.transpose(pt[0:128, 0:HE],
                                        pon[hh][:, li * 128:(li + 1) * 128],
                                        sb_id[0:HE, 0:HE])
                    rcp = fp.tile([128, 1], F32, tag="rcp")
                    nc.vector.reciprocal(rcp[:], pt[:, FOUT:FOUT + 1])
                    nc.vector.tensor_scalar(af[:, hh * FOUT:(hh + 1) * FOUT],
                                            pt[:, 0:FOUT], rcp[:], None, OP.mult)
                # skip connection: x_loc_chunk @ skip_w.T  (+ bias)
                pskip = psf.tile([128, G], F32, tag="pskip")
                nc.tensor.matmul(pskip[:], xTlocr[:, li * 128:(li + 1) * 128],
                                 wskipr, start=True, stop=True)
                nc.vector.tensor_tensor(af[:], af[:], pskip[:], OP.add)
                nc.vector.tensor_tensor(af[:], af[:], sb_biasb[:], OP.add)
                # ELU(z) = max(z,0) + exp(min(z,0)) - 1
                mn = fp.tile([128, G], F32, tag="mn")
                nc.vector.tensor_scalar(mn[:], af[:], 0.0, None, OP.min)
                ex = fp.tile([128, G], F32, tag="ex")
                nc.scalar.activation(ex[:], mn[:], AF.Exp)
                nc.vector.tensor_scalar(af[:], af[:], 0.0, None, OP.max)
                nc.vector.tensor_tensor(af[:], af[:], ex[:], OP.add)
                nc.vector.tensor_scalar(af[:], af[:], -1.0, None, OP.add)
                nc.gpsimd.dma_start(out=d_out[li * 128:(li + 1) * 128, :], in_=af[:])

    _split_multi_waits(nc)
    return nc


def _split_multi_waits(nc):
    """walrus on this toolchain allows only one semaphore-wait command on
    most compute-engine instructions (S3_LW / S3D3_* structs).  Tile's
    scheduler freely emits 2+.  Move all but one wait onto an injected
    same-engine NoOp right before the offending instruction."""
    skip = (mybir.InstEventSemaphore,)
    k = 0
    for f in nc.m.functions:
        for blk in f.blocks:
            new = []
            for ins in blk.instructions:
                si = getattr(ins, "sync_info", None)
                w = list(si.on_wait) if si is not None and si.on_wait else []
                if len(w) > 1 and not isinstance(ins, skip):
                    for wx in w[:-1]:
                        nop = mybir.InstNoOp(name=f"waitsplit-{k}", ins=[], outs=[])
                        nop.engine = ins.engine
                        nop.sync_info = mybir.SyncInfo(on_wait=[wx], on_update=[])
                        new.append(nop)
                        k += 1
                    ins.sync_info = mybir.SyncInfo(on_wait=w[-1:],
                                                   on_update=list(si.on_update))
                new.append(ins)
            blk.instructions[:] = new


_PROG = None


def _get_prog():
    global _PROG
    if _PROG is None:
        _PROG = build_program()
    return _PROG


def make_in_maps(x, mask, proj_param, score_src, score_tgt, skip_w, bias):
    import ml_dtypes
    x = np.asarray(x, np.float32)
    mask = np.asarray(mask, np.float32)
    proj = np.asarray(proj_param, np.float32)
    a_src = np.asarray(score_src, np.float32)[:, :, 0]       # [H, FOUT]
    a_tgt = np.asarray(score_tgt, np.float32)[:, :, 0]
    skip = np.asarray(skip_w, np.float32)
    b = np.asarray(bias, np.float32)

    xT = np.ascontiguousarray(x.T)                           # [128, N]
    wproj = np.ascontiguousarray(proj.transpose(1, 0, 2).reshape(FIN, G))
    w_src = np.einsum('hif,hf->ih', proj, a_src)             # [FIN, H]
    w_tgt = np.einsum('hif,hf->ih', proj, a_tgt)
    wsc = np.concatenate([w_src, w_tgt], 1).astype(np.float32)
    wskip = np.ascontiguousarray(skip.T)                     # [128, G]
    biasb = np.broadcast_to(b[None, :], (128, G)).astype(np.float32)
    mask01 = (mask == 0.0).astype(ml_dtypes.bfloat16)        # [N, N]

    in_maps = []
    for c in range(NCORES):
        r0 = c * NLOC
        cpack = np.ascontiguousarray(np.concatenate(
            [xT, wproj, wsc, wskip, xT[:, r0:r0 + NLOC], biasb], axis=1),
            np.float32)
        sel = np.zeros((4, 4 * 128), ml_dtypes.bfloat16)
        for hh in range(H):
            sel[hh, hh * 128:(hh + 1) * 128] = 1
        in_maps.append({
            "cpack": cpack,
            "mask01": np.ascontiguousarray(mask01[r0:r0 + NLOC, :].T),
            "selc": sel,
        })
    return in_maps


def run(in_maps, trace=False, **kw):
    res = run_bass_kernel_spmd(_get_prog(), in_maps, list(range(NCORES)),
                               trace=trace, **kw)
    out = np.concatenate([res.results[c]["out"] for c in range(NCORES)], axis=0)
    return out, res


def kernel(x, mask, proj_param, score_src, score_tgt, skip_w, bias):
    in_maps = make_in_maps(x, mask, proj_param, score_src, score_tgt, skip_w, bias)
    out, _ = run(in_maps)
    return out.astype(np.float32)
